# revision 5
# baseline (speedup 1.0000x reference)
"""Masked-linear kernel for Trainium2 (8 NeuronCores).

Computes out = data @ (weight * w_mask)^T + bias_p with
  data   [4, 2048, 4096] fp32
  weight [4096, 4096]    fp32
  w_mask [4096, 4096]    fp32
  bias_p [4096]          fp32
  out    [4, 2048, 4096] fp32

Sharding: 2D grid over 8 cores — 4 shards of out-features (N_C=1024) x
2 shards of tokens (M_C=4096). Weight/mask/bias are sliced per n-shard,
data per m-shard; each core computes its [M_C, N_C] output block.

Per core: the masked weight is built on-chip (DVE multiply), transposed
into k-major layout via PE-transpose, and kept resident in SBUF
([128, 32, 1024] = 16.8 MB). Data tiles are PE-transposed per m-tile and
fed as the stationary matmul operand. Matmuls run in float32r mode
(fp32 operands truncated to ~FP22, fp32 accumulate) which streams at
1 cycle/row like bf16. Bias is added during PSUM eviction.
"""

import os
import sys

if "/opt/trn_rl_repo" not in sys.path:
    sys.path.insert(0, "/opt/trn_rl_repo")

import numpy as np

import concourse.bass as bass  # noqa: F401  (import registers bass types)
import concourse.mybir as mybir
import concourse.tile as tile
from concourse import bacc
from concourse.bass_utils import run_bass_kernel_spmd
from concourse.masks import make_identity

# Problem shape (hardcoded per harness contract)
M_TOT = 8192          # 4 * 2048 tokens
K = 4096              # d_in
N_TOT = 4096          # d_out

N_CORES = 8
N_SHARDS = 4          # shards of out-features
M_SHARDS = 2          # shards of tokens
N_C = N_TOT // N_SHARDS   # 1024 out-features per core
M_C = M_TOT // M_SHARDS   # 4096 tokens per core

P = 128
KO = K // P           # 32 k-blocks of 128
MT = M_C // P         # 32 m-tiles of 128 tokens
NH = N_C // 512       # 2 psum-width groups

F32 = mybir.dt.float32

# "f32r" = float32r (FP22-truncated operands, full-rate PE stream);
# "f32"  = true fp32 (4 PE passes, ~4x slower, ~fp32-exact).
DTYPE_MODE = os.environ.get("BASS_KERNEL_DTYPE", "f32r")

LAST_RESULT = None    # BassKernelResults of the most recent run (for test.py)


def _mm_dt():
    return mybir.dt.float32r if DTYPE_MODE == "f32r" else mybir.dt.float32


def _build_program():
    nc = bacc.Bacc("TRN2", target_bir_lowering=False, debug=False,
                   num_devices=N_CORES)

    data_d = nc.dram_tensor("data", [M_C, K], F32, kind="ExternalInput").ap()
    w_d = nc.dram_tensor("w", [N_C, K], F32, kind="ExternalInput").ap()
    mask_d = nc.dram_tensor("mask", [N_C, K], F32, kind="ExternalInput").ap()
    bias_d = nc.dram_tensor("bias", [P, N_C], F32, kind="ExternalInput").ap()
    out_d = nc.dram_tensor("out", [M_C, N_C], F32, kind="ExternalOutput").ap()

    mmdt = _mm_dt()

    with tile.TileContext(nc) as tc:
        with (
            tc.tile_pool(name="const", bufs=1) as const_pool,
            tc.tile_pool(name="wm_res", bufs=1) as wm_res,
            tc.tile_pool(name="wload", bufs=3) as wload,
            tc.tile_pool(name="dnat", bufs=2) as dpool,
            tc.tile_pool(name="dT", bufs=4) as dTpool,
            tc.tile_pool(name="outp", bufs=3) as opool,
            tc.tile_pool(name="pst", bufs=3, space="PSUM") as pst,
            tc.tile_pool(name="psmm", bufs=2, space="PSUM") as psmm,
        ):
            idn = const_pool.tile([P, P], F32, name="idn")
            make_identity(nc, idn)

            bias_sb = const_pool.tile([P, N_C], F32, name="bias_sb")
            nc.sync.dma_start(bias_sb[:], bias_d)

            # Resident masked-weight, k-major: wmT[p=k_in, k_o, n]
            # (float32r dtype: the evicting copy rounds to FP22 on write,
            # which the BIR verifier requires for FP32r matmul operands)
            wmT = wm_res.tile([P, KO, N_C], mmdt, name="wmT")

            # ---- Phase A: build wmT ----
            for no in range(N_C // P):          # 8 blocks of 128 n-rows
                for kc in range(K // 512):      # 8 chunks of 512 k
                    wt = wload.tile([P, 512], F32, name="wt", tag="wt")
                    mt_ = wload.tile([P, 512], F32, name="mt", tag="mt")
                    nc.sync.dma_start(
                        wt[:], w_d[no * P:(no + 1) * P, kc * 512:(kc + 1) * 512])
                    nc.sync.dma_start(
                        mt_[:], mask_d[no * P:(no + 1) * P, kc * 512:(kc + 1) * 512])
                    nc.vector.tensor_mul(wt[:], wt[:], mt_[:])
                    ps = pst.tile([P, 512], F32, name="psA", tag="pst")
                    for j in range(4):
                        nc.tensor.transpose(
                            ps[:, j * P:(j + 1) * P],
                            wt[:, j * P:(j + 1) * P],
                            idn[:],
                        )
                    # psum [k=128, (j, n=128)] -> wmT[:, kc*4+j, no*128:+128]
                    nc.scalar.copy(
                        wmT[:, kc * 4:(kc + 1) * 4, no * P:(no + 1) * P],
                        ps[:].rearrange("p (j n) -> p j n", j=4),
                    )

            # ---- Phase B: stream data, transpose, matmul, evict ----
            for mt in range(MT):
                dnat = dpool.tile([P, K], F32, name="dnat", tag="dnat")
                nc.sync.dma_start(dnat[:], data_d[mt * P:(mt + 1) * P, :])

                pmm = [
                    psmm.tile([P, 512], F32, name=f"pmm{nh}", tag=f"pmm{nh}")
                    for nh in range(NH)
                ]

                for kc in range(KO // 4):       # 8 groups of 4 k-blocks
                    ps = pst.tile([P, 512], F32, name="psB", tag="pst")
                    for j in range(4):
                        ko = kc * 4 + j
                        nc.tensor.transpose(
                            ps[:, j * P:(j + 1) * P],
                            dnat[:, ko * P:(ko + 1) * P],
                            idn[:],
                        )
                    dTt = dTpool.tile([P, 4, P], mmdt, name="dTt", tag="dTt")
                    nc.vector.tensor_copy(
                        dTt[:], ps[:].rearrange("p (j n) -> p j n", j=4))

                    for j in range(4):
                        ko = kc * 4 + j
                        for nh in range(NH):
                            nc.tensor.matmul(
                                pmm[nh][:],
                                dTt[:, j, :],
                                wmT[:, ko, nh * 512:(nh + 1) * 512],
                                start=(ko == 0),
                                stop=(ko == KO - 1),
                            )

                for nh in range(NH):
                    ot = opool.tile([P, 512], F32, name="ot", tag="ot")
                    nc.vector.tensor_add(
                        ot[:], pmm[nh][:], bias_sb[:, nh * 512:(nh + 1) * 512])
                    nc.sync.dma_start(
                        out_d[mt * P:(mt + 1) * P, nh * 512:(nh + 1) * 512], ot[:])

    nc.compile()
    return nc


_PROGRAM = None


def _build_trivial_program():
    nc = bacc.Bacc("TRN2", target_bir_lowering=False, debug=False,
                   num_devices=N_CORES)
    x_d = nc.dram_tensor("x", [P, 256], F32, kind="ExternalInput").ap()
    y_d = nc.dram_tensor("y", [P, 256], F32, kind="ExternalOutput").ap()
    with tile.TileContext(nc) as tc:
        with tc.tile_pool(name="sbuf", bufs=1) as pool:
            t = pool.tile([P, 256], F32, name="t")
            nc.sync.dma_start(t[:], x_d)
            nc.sync.dma_start(y_d, t[:])
    nc.compile()
    return nc


def _time_program_s(nc, reps=10):
    """Min wall-clock of one 8-core dispatch with device-resident inputs."""
    import jax
    from jax.sharding import Mesh, PartitionSpec
    from jax.experimental.shard_map import shard_map
    import time as _time
    from concourse import bass2jax, mybir as _mybir

    bass2jax.install_neuronx_cc_hook()

    in_names, out_names, out_avals, zero_shapes = [], [], [], []
    for alloc in nc.m.functions[0].allocations:
        if not isinstance(_mybir.MemoryLocationSet, type) or not isinstance(
                alloc, _mybir.MemoryLocationSet):
            continue
        name = alloc.memorylocations[0].name
        if alloc.kind == "ExternalInput":
            in_names.append((name, tuple(alloc.tensor_shape),
                             _mybir.dt.np(alloc.dtype)))
        elif alloc.kind == "ExternalOutput":
            out_names.append(name)
            shape = tuple(alloc.tensor_shape)
            dtype = _mybir.dt.np(alloc.dtype)
            out_avals.append(jax.core.ShapedArray(shape, dtype))
            zero_shapes.append((shape, dtype))
    n_params = len(in_names)
    all_names = [n for n, _, _ in in_names] + out_names

    def _body(*args):
        outs = bass2jax._bass_exec_p.bind(
            *args,
            out_avals=tuple(out_avals),
            in_names=tuple(all_names),
            out_names=tuple(out_names),
            lowering_input_output_aliases=(),
            sim_require_finite=True,
            sim_require_nnan=True,
            nc=nc,
        )
        return tuple(outs)

    devices = jax.devices()[:N_CORES]
    mesh = Mesh(np.asarray(devices), ("core",))
    n_all = n_params + len(out_names)
    fn = jax.jit(
        shard_map(_body, mesh=mesh,
                  in_specs=(PartitionSpec("core"),) * n_all,
                  out_specs=(PartitionSpec("core"),) * len(out_names),
                  check_rep=False),
        keep_unused=True,
    )
    sharding = jax.sharding.NamedSharding(mesh, PartitionSpec("core"))
    dev_in = [
        jax.device_put(
            np.zeros((N_CORES * shape[0], *shape[1:]), dtype), sharding)
        for _, shape, dtype in in_names
    ] + [
        jax.device_put(
            np.zeros((N_CORES * shape[0], *shape[1:]), dtype), sharding)
        for shape, dtype in zero_shapes
    ]
    jax.block_until_ready(fn(*dev_in))  # compile + warm
    best = float("inf")
    for _ in range(reps):
        t0 = _time.perf_counter()
        jax.block_until_ready(fn(*dev_in))
        best = min(best, _time.perf_counter() - t0)
    return best


def measure_hw_time_ns(reps=10):
    """HW kernel time estimate: dispatch time minus trivial-NEFF dispatch time."""
    global _PROGRAM
    if _PROGRAM is None:
        _PROGRAM = _build_program()
    t_kernel = _time_program_s(_PROGRAM, reps)
    t_floor = _time_program_s(_build_trivial_program(), reps)
    print(f"[timing] dispatch: kernel {t_kernel*1e3:.3f} ms, "
          f"trivial {t_floor*1e3:.3f} ms")
    return int((t_kernel - t_floor) * 1e9)


def kernel(data, weight, w_mask, bias_p):
    global _PROGRAM, LAST_RESULT
    data = np.asarray(data, dtype=np.float32)
    weight = np.asarray(weight, dtype=np.float32)
    w_mask = np.asarray(w_mask, dtype=np.float32)
    bias_p = np.asarray(bias_p, dtype=np.float32)

    dataf = np.ascontiguousarray(data.reshape(M_TOT, K))

    if _PROGRAM is None:
        _PROGRAM = _build_program()
    nc = _PROGRAM

    in_maps = []
    for c in range(N_CORES):
        ns = c % N_SHARDS
        ms = c // N_SHARDS
        in_maps.append({
            "data": np.ascontiguousarray(dataf[ms * M_C:(ms + 1) * M_C]),
            "w": np.ascontiguousarray(weight[ns * N_C:(ns + 1) * N_C]),
            "mask": np.ascontiguousarray(w_mask[ns * N_C:(ns + 1) * N_C]),
            "bias": np.ascontiguousarray(
                np.tile(bias_p[ns * N_C:(ns + 1) * N_C][None, :], (P, 1))),
        })

    res = run_bass_kernel_spmd(nc, in_maps, core_ids=list(range(N_CORES)))
    LAST_RESULT = res

    out = np.empty((M_TOT, N_TOT), dtype=np.float32)
    for c in range(N_CORES):
        ns = c % N_SHARDS
        ms = c // N_SHARDS
        out[ms * M_C:(ms + 1) * M_C, ns * N_C:(ns + 1) * N_C] = \
            res.results[c]["out"]
    return out.reshape(4, 2048, N_TOT)
